# revision 6
# baseline (speedup 1.0000x reference)
"""Trainium2 Bass kernel for nn_MultiHeadAttention_5162550690632 (v2).

B=2, S=2048, EMB=1024, H=16 heads x 64 dim. Sharding: 8 cores =
2 batches x 4 head-groups (4 heads each); every shard is independent
(tensor parallel on heads + data parallel on batch), no collectives.

v2 changes vs baseline (275.9us):
  * All operands bf16 (PSUM stays fp32, biases fp32): halves input DMA
    (x 16MB->8MB, w 3MB->1.5MB) and SBUF; host-sim rel err 7.7e-3 vs
    the 2e-2 gate.
  * K projection runs e-major across all 4 sq blocks (8 concurrent
    PSUM accumulation groups) so the first matmul starts as soon as the
    first 192KB of wk/xk land instead of after ~6MB (33us startup stall
    in the baseline trace).
  * kd (block-diagonal K tiles for the scores matmul) is written
    directly from the K-proj PSUM drain: one bias-fused DVE copy per
    (sq,c) plus 4 strided SBUF->SBUF DMAs, replacing 192 DVE casts
    (~48us of DVE busy) in the baseline.
  * Attention is software-pipelined immediately after K+Q(0): scores
    (sq0,h0) starts ~33us in; the 16 V-projection chains and Q(sq1..3)
    are interleaved into the scores/AV pipeline so the PE never idles
    while ACT (exp) streams.
  * Per-head normalize batches the 4 transposes into one PSUM tile and
    uses a single strided reciprocal.
"""

import numpy as np

import concourse.bass as bass
import concourse.mybir as mybir
import concourse.tile as tile
from concourse.tile import ScopedClock
from concourse.bass_utils import run_bass_kernel_spmd
from concourse.masks import make_identity

# ---------------------------------------------------------------------------
# Workaround: this neuronxcc rejects >1 sync wait on several instruction
# encodings ("Too many sync wait commands", CoreV3GenImpl setupSyncWait).
# TileContext attaches multiple waits per instruction and its exit drain
# waits on every live processor.  Split every extra wait into a dedicated
# single-wait NOP on the same engine right before the instruction —
# per-engine queues are in-order, so this is semantically identical.

_MAX_WAITS = 1


def _legalize_multi_waits(tc):
    nc = tc.nc
    for fn in nc.m.functions:
        for bb in fn.blocks:
            snapshot = list(bb.instructions)
            if not any(
                inst.sync_info is not None
                and len(inst.sync_info.on_wait) > _MAX_WAITS
                for inst in snapshot
            ):
                continue
            created = []
            new_list = []
            for inst in snapshot:
                si = inst.sync_info
                if si is not None and len(si.on_wait) > _MAX_WAITS:
                    waits = list(si.on_wait)
                    for w in waits[_MAX_WAITS:]:
                        nop = nc.engines[inst.engine].nop(
                            nofuse=True, hint="wait_split"
                        )
                        nop.ins.sync_info = mybir.SyncInfo(
                            on_wait=[w], on_update=[]
                        )
                        created.append(nop.ins.name)
                        new_list.append(nop.ins)
                    inst.sync_info = mybir.SyncInfo(
                        on_wait=waits[:_MAX_WAITS], on_update=list(si.on_update)
                    )
                new_list.append(inst)
            cur = nc.cur_bb.bb if hasattr(nc.cur_bb, "bb") else nc.cur_bb
            if cur is not None and cur.name != bb.name:
                cur.instructions = [
                    i for i in cur.instructions if i.name not in created
                ]
            bb.instructions = new_list


def _patched_drain_and_barrier(self, tick_clock, wait_clock):
    nc = self.nc
    probe = nc.sync.nop(nofuse=True, hint="drain_probe")
    wait_clock.add_sem_waits(probe.ins, ScopedClock({None: tick_clock.global_clock}))
    waits = list(probe.ins.sync_info.on_wait)
    probe.ins.sync_info = mybir.SyncInfo(on_wait=[], on_update=[])
    name2sem = {s.name: s for s in self.sems.allocated().values()}
    for w in waits:
        nc.sync.wait_ge(name2sem[w.ant_name], w.wait_value)
    _legalize_multi_waits(self)
    nc.sync.drain()
    nc.all_engine_barrier()
    popped = nc._tile_sem_poison_stack.pop()
    assert popped is self._sem_poison
    nc.clear_and_free_semaphores(list(self.sems.allocated().values()))
    nc.all_engine_barrier()


tile.TileContext._drain_and_barrier = _patched_drain_and_barrier

# ---------------------------------------------------------------------------

F32 = mybir.dt.float32
BF16 = mybir.dt.bfloat16
AF = mybir.ActivationFunctionType
ALU = mybir.AluOpType

B, S, EMB = 2, 2048, 1024
H, DH = 16, 64
NCORES = 8
HG = 4                      # head-groups
NH = H // HG                # heads per core = 4
CH = NH * DH                # channels per core = 256
EC = EMB // 128             # EMB chunks = 8
SQT = 512                   # q-tile width
NSQ = S // SQT              # 4
NSK = S // 128              # 16 sk chunks


def _build_nc():
    nc = bass.Bass()

    xqT = nc.declare_dram_parameter("xqT", [128, NSQ, EC, SQT], BF16, isOutput=False)
    xkT = nc.declare_dram_parameter("xkT", [128, EC, S], BF16, isOutput=False)
    wqT = nc.declare_dram_parameter("wqT", [128, EC, CH], BF16, isOutput=False)
    wkT = nc.declare_dram_parameter("wkT", [128, EC, CH], BF16, isOutput=False)
    wvT = nc.declare_dram_parameter("wvT", [128, EC, CH], BF16, isOutput=False)
    bqc = nc.declare_dram_parameter("bqc", [128, 2], F32, isOutput=False)
    bkc = nc.declare_dram_parameter("bkc", [128, 2], F32, isOutput=False)
    bv = nc.declare_dram_parameter("bv", [1, CH], BF16, isOutput=False)
    maskT = nc.declare_dram_parameter("maskT", [128, NSQ, NSK, SQT], BF16, isOutput=False)
    ones_row = nc.declare_dram_parameter("ones_row", [1, 128], BF16, isOutput=False)
    ones_col = nc.declare_dram_parameter(
        "ones_col", [128, NSK, NH], BF16, isOutput=False
    )
    zkd = nc.declare_dram_parameter("zkd", [128, NH * 4 * 128], BF16, isOutput=False)
    out = nc.declare_dram_parameter("out", [NSQ, 4, 128, CH], F32, isOutput=True)

    with tile.TileContext(nc) as tc:
        with (
            tc.tile_pool(name="persist", bufs=1) as persist,
            tc.tile_pool(name="xqp", bufs=3) as xqp,
            tc.tile_pool(name="qt2p", bufs=2) as qt2p,
            tc.tile_pool(name="maskp", bufs=4) as maskp,
            tc.tile_pool(name="probsp", bufs=3) as probsp,
            tc.tile_pool(name="kcp", bufs=8) as kcp,
            tc.tile_pool(name="escp", bufs=3) as escp,
            tc.tile_pool(name="zaugp", bufs=2) as zaugp,
            tc.tile_pool(name="zp", bufs=2) as zp,
            tc.tile_pool(name="recipp", bufs=2) as recipp,
        ):
            # block-diag K tiles: kd[g][:, h*4+tl, :] covers sk block 4g+tl
            kd = [
                persist.tile(
                    [128, NH * 4, 128], BF16, tag=f"kd_{g}", name=f"kd_{g}"
                )
                for g in range(NSQ)
            ]
            vaug_sb = [
                persist.tile(
                    [128, 4, NH, DH + 1], BF16, tag=f"vaug_{g}", name=f"vaug_{g}"
                )
                for g in range(NSQ)
            ]
            wq_sb = persist.tile([128, EC, CH], BF16, tag="wq")
            wk_sb = persist.tile([128, EC, CH], BF16, tag="wk")
            wv_sb = persist.tile([128, EC, CH], BF16, tag="wv")
            xk_all = persist.tile([128, EC, S], BF16, tag="xk_all")
            ident = persist.tile([128, 128], F32, tag="ident")
            ones_r = persist.tile([1, 128], BF16, tag="ones_r")
            bq_sb = persist.tile([128, 2], F32, tag="bq")
            bk_sb = persist.tile([128, 2], F32, tag="bk")
            bv_sb = persist.tile([1, CH], BF16, tag="bv")

            make_identity(nc, ident[:])
            # smalls + zero-fills on the gpsimd queue
            nc.gpsimd.dma_start(ones_r[:], ones_row[:])
            nc.gpsimd.dma_start(bq_sb[:], bqc[:])
            nc.gpsimd.dma_start(bk_sb[:], bkc[:])
            nc.gpsimd.dma_start(bv_sb[:], bv[:])
            for g in range(NSQ):
                nc.gpsimd.dma_start(
                    kd[g].rearrange("p i x -> p (i x)"), zkd[:]
                )
                nc.gpsimd.dma_start(
                    vaug_sb[g][:, :, :, DH : DH + 1],
                    ones_col[:, 4 * g : 4 * g + 4, :, None],
                )

            xq_t = [
                xqp.tile([128, EC, SQT], BF16, tag="xq", name=f"xq_{sq}")
                for sq in range(NSQ)
            ]
            mask_t = {}

            def mask_dma(sq, half, eng):
                key = (sq, half)
                if key not in mask_t:
                    mask_t[key] = maskp.tile(
                        [128, NSK // 2, SQT],
                        BF16,
                        tag="mask",
                        name=f"mask_{sq}_{half}",
                    )
                lo = half * (NSK // 2)
                eng.dma_start(
                    mask_t[key][:], maskT[:, sq, lo : lo + NSK // 2, :]
                )

            # ---------------- K projection, e-major ----------------
            with tc.tile_pool(name="ps_k", bufs=1, space="PSUM") as ps_k:
                kps = [
                    [
                        ps_k.tile(
                            [128, SQT], F32, tag=f"kps_{sq}_{c}", name=f"kps_{sq}_{c}"
                        )
                        for c in range(2)
                    ]
                    for sq in range(NSQ)
                ]
                # sync queue: few big DMAs, priority order.  masks 0/1 go
                # on the gpsimd queue (idle early) so the first mults never
                # wait; masks 2/3 trail on sync behind the xq blocks.
                nc.sync.dma_start(wk_sb[:], wkT[:])
                for e in range(EC):
                    nc.sync.dma_start(xk_all[:, e, :], xkT[:, e, :])
                nc.sync.dma_start(wq_sb[:], wqT[:])
                nc.sync.dma_start(xq_t[0][:], xqT[:, 0, :, :])
                nc.sync.dma_start(wv_sb[:], wvT[:])
                mask_dma(0, 0, nc.gpsimd)
                mask_dma(0, 1, nc.gpsimd)
                mask_dma(1, 0, nc.gpsimd)
                mask_dma(1, 1, nc.gpsimd)
                for sq in range(1, NSQ):
                    nc.sync.dma_start(xq_t[sq][:], xqT[:, sq, :, :])
                for sq in (2, 3):
                    mask_dma(sq, 0, nc.sync)
                    mask_dma(sq, 1, nc.sync)

                for e in range(EC):
                    for sq in range(NSQ):
                        ssl = slice(sq * SQT, (sq + 1) * SQT)
                        for c in range(2):
                            nc.tensor.matmul(
                                kps[sq][c][:],
                                wk_sb[:, e, c * 128 : (c + 1) * 128],
                                xk_all[:, e, ssl],
                                start=(e == 0),
                                stop=(e == EC - 1),
                            )

                # drain: bias-fused DVE stage per (sq,c); aligned diag
                # halves via DVE copy; partition-shifted halves via
                # SBUF->SBUF DMA, head-ordered and split over the gpsimd and
                # scalar DMA queues so head-0 tiles land first
                kcs = {}

                def emit_kc(sq, c):
                    t_ = kcp.tile([128, SQT], BF16, tag="kc", name=f"kc_{sq}_{c}")
                    kcs[sq, c] = t_
                    nc.vector.tensor_scalar_add(
                        t_[:], kps[sq][c][:], bk_sb[:, c : c + 1]
                    )

                def kd_aligned(c):
                    hA, hB = 2 * c, 2 * c + 1
                    for sq in range(NSQ):
                        kcv = kcs[sq, c].rearrange("p (t x) -> p t x", t=4)
                        nc.vector.tensor_copy(
                            kd[sq][0:64, hA * 4 : hA * 4 + 4, 0:64],
                            kcv[0:64, :, 0:64],
                        )
                        nc.vector.tensor_copy(
                            kd[sq][64:128, hB * 4 : hB * 4 + 4, 64:128],
                            kcv[64:128, :, 64:128],
                        )

                def kd_shifted(c):
                    hA, hB = 2 * c, 2 * c + 1
                    for i, sq in enumerate(range(NSQ)):
                        kcv = kcs[sq, c].rearrange("p (t x) -> p t x", t=4)
                        eng = nc.gpsimd if i % 2 == 0 else nc.scalar
                        eng.dma_start(
                            kd[sq][64:128, hA * 4 : hA * 4 + 4, 64:128],
                            kcv[0:64, :, 64:128],
                        )
                    for i, sq in enumerate(range(NSQ)):
                        kcv = kcs[sq, c].rearrange("p (t x) -> p t x", t=4)
                        eng = nc.scalar if i % 2 == 0 else nc.gpsimd
                        eng.dma_start(
                            kd[sq][0:64, hB * 4 : hB * 4 + 4, 0:64],
                            kcv[64:128, :, 0:64],
                        )

                for sq in range(NSQ):
                    emit_kc(sq, 0)
                for sq in range(NSQ):
                    emit_kc(sq, 1)
                kd_shifted(0)
                kd_aligned(0)
                kd_shifted(1)

                # Q0 recycles the sq0 K-chain PSUM tags so it can start as
                # soon as those chains drain, inside the ps_k scope
                qt2_first = [None]

                def emit_q0():
                    qt2 = qt2p.tile([128, NH, SQT], BF16, tag="qt2", name="qt2_0")
                    qt2_first[0] = qt2
                    for c in range(2):
                        ps = ps_k.tile(
                            [128, SQT], F32, tag=f"kps_0_{c}", name=f"q0ps_{c}"
                        )
                        for e in range(EC):
                            nc.tensor.matmul(
                                ps[:],
                                wq_sb[:, e, c * 128 : (c + 1) * 128],
                                xq_t[0][:, e, :],
                                start=(e == 0),
                                stop=(e == EC - 1),
                            )
                        qtc = kcp.tile([128, SQT], BF16, tag="kc", name="qtc0")
                        nc.vector.tensor_scalar_add(
                            qtc[:], ps[:], bq_sb[:, c : c + 1]
                        )
                        hA, hB = 2 * c, 2 * c + 1
                        nc.sync.dma_start(qt2[0:64, hA, :], qtc[0:64, :])
                        nc.sync.dma_start(qt2[64:128, hA, :], qtc[0:64, :])
                        nc.sync.dma_start(qt2[0:64, hB, :], qtc[64:128, :])
                        nc.sync.dma_start(qt2[64:128, hB, :], qtc[64:128, :])

                emit_q0()
                kd_aligned(1)

            # ---------------- attention + V/Q interleaved ----------------
            with (
                tc.tile_pool(name="ps_pq", bufs=2, space="PSUM") as ps_pq,
                tc.tile_pool(name="ps_s", bufs=2, space="PSUM") as ps_s,
                tc.tile_pool(name="ps_z", bufs=1, space="PSUM") as ps_z,
                tc.tile_pool(name="ps_t", bufs=1, space="PSUM") as ps_t,
            ):
                def emit_q(sq):
                    qt2 = qt2p.tile(
                        [128, NH, SQT], BF16, tag="qt2", name=f"qt2_{sq}"
                    )
                    for c in range(2):
                        ps = ps_pq.tile([128, SQT], F32, tag="projq", name="psq")
                        for e in range(EC):
                            nc.tensor.matmul(
                                ps[:],
                                wq_sb[:, e, c * 128 : (c + 1) * 128],
                                xq_t[sq][:, e, :],
                                start=(e == 0),
                                stop=(e == EC - 1),
                            )
                        qtc = kcp.tile([128, SQT], BF16, tag="kc", name="qtc")
                        nc.vector.tensor_scalar_add(
                            qtc[:], ps[:], bq_sb[:, c : c + 1]
                        )
                        hA, hB = 2 * c, 2 * c + 1
                        nc.sync.dma_start(qt2[0:64, hA, :], qtc[0:64, :])
                        nc.sync.dma_start(qt2[64:128, hA, :], qtc[0:64, :])
                        nc.sync.dma_start(qt2[0:64, hB, :], qtc[64:128, :])
                        nc.sync.dma_start(qt2[64:128, hB, :], qtc[64:128, :])
                    return qt2

                def emit_v(sq, j):
                    jsl = slice(sq * SQT + j * 128, sq * SQT + (j + 1) * 128)
                    ps = ps_s.tile([128, 2 * SQT], F32, tag="sc", name="psv")
                    psv = ps.rearrange("p (a c) -> p a c", c=CH)
                    for e in range(EC):
                        nc.tensor.matmul(
                            psv[:, 0, :],
                            xk_all[:, e, jsl],
                            wv_sb[:, e, :],
                            start=(e == 0),
                            stop=False,
                        )
                    nc.tensor.matmul(
                        psv[:, 0, :], ones_r[:], bv_sb[:], start=False, stop=True
                    )
                    nc.vector.tensor_copy(
                        vaug_sb[sq][:, j, :, 0:DH],
                        ps.rearrange("p (x h d) -> p x h d", h=NH, d=DH)[:, 0, :, :],
                    )

                def emit_scores(sq, h, qt2, probs):
                    # 2-chunk tiles: [128, 1024] 2D APs amortize ACT/DVE
                    # per-op overhead; all-bf16 multiply hits the DVE 2x mode
                    probs_v = probs.rearrange("p (a u) q -> p a (u q)", u=2)
                    mask_h = [
                        mask_t[(sq, 0)].rearrange("p (a u) q -> p a (u q)", u=2),
                        mask_t[(sq, 1)].rearrange("p (a u) q -> p a (u q)", u=2),
                    ]
                    for t2 in range(NSK // 2):
                        ps = ps_s.tile([128, 2 * SQT], F32, tag="sc", name="pssc")
                        for u in range(2):
                            t = 2 * t2 + u
                            nc.tensor.matmul(
                                ps[:, u * SQT : (u + 1) * SQT],
                                kd[t // 4][:, h * 4 + t % 4, :],
                                qt2[:, h, :],
                                start=True,
                                stop=True,
                            )
                        esc = escp.tile([128, 2 * SQT], BF16, tag="esc", name="esc")
                        nc.scalar.activation(esc[:], ps[:], AF.Exp, scale=0.125)
                        nc.vector.tensor_tensor(
                            probs_v[:, t2, :],
                            esc[:],
                            mask_h[t2 // 4][:, t2 % 4, :],
                            ALU.mult,
                        )

                def emit_av(sq, h, z_sb, probs):
                    zps = ps_z.tile([DH + 1, SQT], F32, tag="zps", name="zps")
                    for t in range(NSK):
                        nc.tensor.matmul(
                            zps[:],
                            vaug_sb[t // 4][:, t % 4, h, :],
                            probs[:, t, :],
                            start=(t == 0),
                            stop=(t == NSK - 1),
                        )
                    zaug = zaugp.tile([DH + 1, SQT], F32, tag="zaug", name="zaug")
                    nc.vector.tensor_copy(zaug[:], zps[:])
                    tps = ps_t.tile([128, 4, DH + 1], F32, tag="tps", name="tps")
                    for j in range(4):
                        nc.tensor.transpose(
                            tps[:, j, :],
                            zaug[:, j * 128 : (j + 1) * 128],
                            ident[: DH + 1, : DH + 1],
                        )
                    recip = recipp.tile([128, 4], F32, tag="recip", name="recip")
                    nc.vector.reciprocal(recip[:], tps[:, :, DH])
                    for j in range(4):
                        nc.vector.tensor_scalar_mul(
                            z_sb[:, j, h * DH : (h + 1) * DH],
                            tps[:, j, 0:DH],
                            recip[:, j : j + 1],
                        )

                # software pipeline: head h's scores ahead of head h-1's AV;
                # V-proj chains slot in after the first two score blocks and
                # Q(sq+1) after each sq's last score block.
                qt2_cur = qt2_first[0]
                prev = None
                for sq in range(NSQ):
                    z_sb = zp.tile([128, 4, CH], F32, tag="z", name="z_sb")
                    for h in range(NH):
                        probs = probsp.tile(
                            [128, NSK, SQT], BF16, tag="probs", name=f"probs_{sq}_{h}"
                        )
                        emit_scores(sq, h, qt2_cur, probs)
                        if sq == 0 and h == 0:
                            for s2 in (0, 1):
                                for j in range(4):
                                    emit_v(s2, j)
                        elif sq == 0 and h == 1:
                            for s2 in (2, 3):
                                for j in range(4):
                                    emit_v(s2, j)
                        if prev is not None:
                            emit_av(*prev)
                            if prev[1] == NH - 1:
                                psq, pz = prev[0], prev[2]
                                nc.gpsimd.dma_start(
                                    out[psq].rearrange("j p c -> p j c"), pz[:]
                                )
                        prev = (sq, h, z_sb, probs)
                    if sq + 1 < NSQ:
                        qt2_next = emit_q(sq + 1)
                        qt2_cur = qt2_next
                emit_av(*prev)
                nc.gpsimd.dma_start(
                    out[prev[0]].rearrange("j p c -> p j c"), prev[2][:]
                )

    return nc


_NC_CACHE = {}


def _get_nc():
    if "nc" not in _NC_CACHE:
        _NC_CACHE["nc"] = _build_nc()
    return _NC_CACHE["nc"]


def _prep_in_maps(x_q, x_k_v, attn_mask, w_q, b_q, w_k, b_k, w_v, b_v):
    import ml_dtypes

    bf16 = ml_dtypes.bfloat16
    x_q = np.asarray(x_q, dtype=np.float32)
    x_k_v = np.asarray(x_k_v, dtype=np.float32)
    attn_mask = np.asarray(attn_mask)
    w_q = np.asarray(w_q, dtype=np.float32)
    w_k = np.asarray(w_k, dtype=np.float32)
    w_v = np.asarray(w_v, dtype=np.float32)
    b_q = np.asarray(b_q, dtype=np.float32)
    b_k = np.asarray(b_k, dtype=np.float32)
    b_v = np.asarray(b_v, dtype=np.float32)

    # p-major layouts: every device DMA reads a contiguous per-partition
    # block (small strided segments run at ~35GB/s vs ~330GB/s contiguous)
    xqT = [
        np.ascontiguousarray(
            x_q[b].T.reshape(EC, 128, NSQ, SQT).transpose(1, 2, 0, 3)
        ).astype(bf16)
        for b in range(B)
    ]
    xkT = [
        np.ascontiguousarray(x_k_v[b].T.reshape(EC, 128, S).transpose(1, 0, 2))
        .astype(bf16)
        for b in range(B)
    ]
    maskT = [
        np.ascontiguousarray(
            (~attn_mask[b]).T.reshape(NSK, 128, NSQ, SQT).transpose(1, 2, 0, 3)
        ).astype(bf16)
        for b in range(B)
    ]
    def _wprep(w, g):
        wt = w[g * CH : (g + 1) * CH].T.reshape(EC, 128, CH).transpose(1, 0, 2)
        return np.ascontiguousarray(wt).astype(bf16)
    wqT = [_wprep(w_q, g) for g in range(HG)]
    wkT = [_wprep(w_k, g) for g in range(HG)]
    wvT = [_wprep(w_v, g) for g in range(HG)]
    # bias columns [128, 2]: bqc[p, c] = b_q[g*CH + c*128 + p]
    bqc = [
        np.ascontiguousarray(b_q[g * CH : (g + 1) * CH].reshape(2, 128).T)
        for g in range(HG)
    ]
    bkc = [
        np.ascontiguousarray(b_k[g * CH : (g + 1) * CH].reshape(2, 128).T)
        for g in range(HG)
    ]
    bvs = [
        b_v[g * CH : (g + 1) * CH].reshape(1, CH).astype(bf16) for g in range(HG)
    ]
    ones_row = np.ones((1, 128), dtype=bf16)
    ones_col = np.ones((128, NSK, NH), dtype=bf16)
    zkd = np.zeros((128, NH * 4 * 128), dtype=bf16)

    in_maps = []
    for core in range(NCORES):
        b, g = divmod(core, HG)
        in_maps.append(
            {
                "xqT": xqT[b],
                "xkT": xkT[b],
                "maskT": maskT[b],
                "wqT": wqT[g],
                "wkT": wkT[g],
                "wvT": wvT[g],
                "bqc": bqc[g],
                "bkc": bkc[g],
                "bv": bvs[g],
                "ones_row": ones_row,
                "ones_col": ones_col,
                "zkd": zkd,
            }
        )
    return in_maps


def _run(inputs, **runner_kwargs):
    nc = _get_nc()
    in_maps = _prep_in_maps(**inputs)
    res = run_bass_kernel_spmd(nc, in_maps, list(range(NCORES)), **runner_kwargs)
    z = np.empty((B, S, H * DH), dtype=np.float32)
    for core in range(NCORES):
        b, g = divmod(core, HG)
        z[b, :, g * CH : (g + 1) * CH] = res.results[core]["out"].reshape(S, CH)
    return z, res


def kernel(**inputs) -> np.ndarray:
    z, _ = _run(inputs)
    return z


# revision 7
# speedup vs baseline: 1.0289x; 1.0289x over previous
"""Trainium2 Bass kernel for nn_MultiHeadAttention_5162550690632 (v2).

B=2, S=2048, EMB=1024, H=16 heads x 64 dim. Sharding: 8 cores =
2 batches x 4 head-groups (4 heads each); every shard is independent
(tensor parallel on heads + data parallel on batch), no collectives.

v2 changes vs baseline (275.9us):
  * All operands bf16 (PSUM stays fp32, biases fp32): halves input DMA
    (x 16MB->8MB, w 3MB->1.5MB) and SBUF; host-sim rel err 7.7e-3 vs
    the 2e-2 gate.
  * K projection runs e-major across all 4 sq blocks (8 concurrent
    PSUM accumulation groups) so the first matmul starts as soon as the
    first 192KB of wk/xk land instead of after ~6MB (33us startup stall
    in the baseline trace).
  * kd (block-diagonal K tiles for the scores matmul) is written
    directly from the K-proj PSUM drain: one bias-fused DVE copy per
    (sq,c) plus 4 strided SBUF->SBUF DMAs, replacing 192 DVE casts
    (~48us of DVE busy) in the baseline.
  * Attention is software-pipelined immediately after K+Q(0): scores
    (sq0,h0) starts ~33us in; the 16 V-projection chains and Q(sq1..3)
    are interleaved into the scores/AV pipeline so the PE never idles
    while ACT (exp) streams.
  * Per-head normalize batches the 4 transposes into one PSUM tile and
    uses a single strided reciprocal.
"""

import numpy as np

import concourse.bass as bass
import concourse.mybir as mybir
import concourse.tile as tile
from concourse.tile import ScopedClock
from concourse.bass_utils import run_bass_kernel_spmd
from concourse.masks import make_identity

# ---------------------------------------------------------------------------
# Workaround: this neuronxcc rejects >1 sync wait on several instruction
# encodings ("Too many sync wait commands", CoreV3GenImpl setupSyncWait).
# TileContext attaches multiple waits per instruction and its exit drain
# waits on every live processor.  Split every extra wait into a dedicated
# single-wait NOP on the same engine right before the instruction —
# per-engine queues are in-order, so this is semantically identical.

_MAX_WAITS = 1


def _legalize_multi_waits(tc):
    nc = tc.nc
    for fn in nc.m.functions:
        for bb in fn.blocks:
            snapshot = list(bb.instructions)
            if not any(
                inst.sync_info is not None
                and len(inst.sync_info.on_wait) > _MAX_WAITS
                for inst in snapshot
            ):
                continue
            created = []
            new_list = []
            for inst in snapshot:
                si = inst.sync_info
                if si is not None and len(si.on_wait) > _MAX_WAITS:
                    waits = list(si.on_wait)
                    for w in waits[_MAX_WAITS:]:
                        nop = nc.engines[inst.engine].nop(
                            nofuse=True, hint="wait_split"
                        )
                        nop.ins.sync_info = mybir.SyncInfo(
                            on_wait=[w], on_update=[]
                        )
                        created.append(nop.ins.name)
                        new_list.append(nop.ins)
                    inst.sync_info = mybir.SyncInfo(
                        on_wait=waits[:_MAX_WAITS], on_update=list(si.on_update)
                    )
                new_list.append(inst)
            cur = nc.cur_bb.bb if hasattr(nc.cur_bb, "bb") else nc.cur_bb
            if cur is not None and cur.name != bb.name:
                cur.instructions = [
                    i for i in cur.instructions if i.name not in created
                ]
            bb.instructions = new_list


def _patched_drain_and_barrier(self, tick_clock, wait_clock):
    nc = self.nc
    probe = nc.sync.nop(nofuse=True, hint="drain_probe")
    wait_clock.add_sem_waits(probe.ins, ScopedClock({None: tick_clock.global_clock}))
    waits = list(probe.ins.sync_info.on_wait)
    probe.ins.sync_info = mybir.SyncInfo(on_wait=[], on_update=[])
    name2sem = {s.name: s for s in self.sems.allocated().values()}
    for w in waits:
        nc.sync.wait_ge(name2sem[w.ant_name], w.wait_value)
    _legalize_multi_waits(self)
    nc.sync.drain()
    nc.all_engine_barrier()
    popped = nc._tile_sem_poison_stack.pop()
    assert popped is self._sem_poison
    nc.clear_and_free_semaphores(list(self.sems.allocated().values()))
    nc.all_engine_barrier()


tile.TileContext._drain_and_barrier = _patched_drain_and_barrier

# ---------------------------------------------------------------------------

F32 = mybir.dt.float32
BF16 = mybir.dt.bfloat16
AF = mybir.ActivationFunctionType
ALU = mybir.AluOpType

B, S, EMB = 2, 2048, 1024
H, DH = 16, 64
NCORES = 8
HG = 4                      # head-groups
NH = H // HG                # heads per core = 4
CH = NH * DH                # channels per core = 256
EC = EMB // 128             # EMB chunks = 8
SQT = 512                   # q-tile width
NSQ = S // SQT              # 4
NSK = S // 128              # 16 sk chunks


def _build_nc():
    nc = bass.Bass()

    xqT = nc.declare_dram_parameter("xqT", [128, NSQ, EC, SQT], BF16, isOutput=False)
    xkT = nc.declare_dram_parameter("xkT", [128, EC, S], BF16, isOutput=False)
    wqT = nc.declare_dram_parameter("wqT", [128, EC, CH], BF16, isOutput=False)
    wkT = nc.declare_dram_parameter("wkT", [128, EC, CH], BF16, isOutput=False)
    wvT = nc.declare_dram_parameter("wvT", [128, EC, CH], BF16, isOutput=False)
    bqc = nc.declare_dram_parameter("bqc", [128, 2], F32, isOutput=False)
    bkc = nc.declare_dram_parameter("bkc", [128, 2], F32, isOutput=False)
    bv = nc.declare_dram_parameter("bv", [1, CH], BF16, isOutput=False)
    maskT = nc.declare_dram_parameter("maskT", [128, NSQ, NSK, SQT], BF16, isOutput=False)
    ones_row = nc.declare_dram_parameter("ones_row", [1, 128], BF16, isOutput=False)
    ones_col = nc.declare_dram_parameter(
        "ones_col", [128, NSK, NH], BF16, isOutput=False
    )
    zkd = nc.declare_dram_parameter("zkd", [128, NH * 4 * 128], BF16, isOutput=False)
    out = nc.declare_dram_parameter("out", [NSQ, 4, 128, CH], F32, isOutput=True)

    with tile.TileContext(nc) as tc:
        with (
            tc.tile_pool(name="persist", bufs=1) as persist,
            tc.tile_pool(name="xqp", bufs=2) as xqp,
            tc.tile_pool(name="qt2p", bufs=2) as qt2p,
            tc.tile_pool(name="maskp", bufs=4) as maskp,
            tc.tile_pool(name="probsp", bufs=3) as probsp,
            tc.tile_pool(name="kcp", bufs=2) as kcp,
            tc.tile_pool(name="ktcp", bufs=8) as ktcp,
            tc.tile_pool(name="escp", bufs=2) as escp,
            tc.tile_pool(name="zaugp", bufs=2) as zaugp,
            tc.tile_pool(name="zp", bufs=2) as zp,
            tc.tile_pool(name="recipp", bufs=2) as recipp,
        ):
            # block-diag K tiles: kd[g][:, h*4+tl, :] covers sk block 4g+tl
            kd = [
                persist.tile(
                    [128, NH * 4, 128], BF16, tag=f"kd_{g}", name=f"kd_{g}"
                )
                for g in range(NSQ)
            ]
            vaug_sb = [
                persist.tile(
                    [128, 4, NH, DH + 1], BF16, tag=f"vaug_{g}", name=f"vaug_{g}"
                )
                for g in range(NSQ)
            ]
            wq_sb = persist.tile([128, EC, CH], BF16, tag="wq")
            wk_sb = persist.tile([128, EC, CH], BF16, tag="wk")
            wv_sb = persist.tile([128, EC, CH], BF16, tag="wv")
            xk_all = persist.tile([128, EC, S], BF16, tag="xk_all")
            ident = persist.tile([128, 128], F32, tag="ident")
            ones_r = persist.tile([1, 128], BF16, tag="ones_r")
            bq_sb = persist.tile([128, 2], F32, tag="bq")
            bk_sb = persist.tile([128, 2], F32, tag="bk")
            bv_sb = persist.tile([1, CH], BF16, tag="bv")

            make_identity(nc, ident[:])
            # warm the Exp activation table during startup so the first real
            # exp doesn't pay the ~1.3us ACT_TABLE_LOAD on the critical path
            warm = persist.tile([1, 8], BF16, tag="warm")
            nc.scalar.activation(warm[:], ident[0:1, 0:8], AF.Exp, scale=0.125)
            # smalls + zero-fills on the gpsimd queue
            nc.gpsimd.dma_start(ones_r[:], ones_row[:])
            nc.gpsimd.dma_start(bq_sb[:], bqc[:])
            nc.gpsimd.dma_start(bk_sb[:], bkc[:])
            nc.gpsimd.dma_start(bv_sb[:], bv[:])
            for g in range(NSQ):
                nc.gpsimd.dma_start(
                    kd[g].rearrange("p i x -> p (i x)"), zkd[:]
                )
                nc.gpsimd.dma_start(
                    vaug_sb[g][:, :, :, DH : DH + 1],
                    ones_col[:, 4 * g : 4 * g + 4, :, None],
                )

            xq_t = [
                xqp.tile([128, EC, SQT], BF16, tag="xq", name=f"xq_{sq}")
                for sq in range(NSQ)
            ]
            mask_t = {}

            def mask_dma(sq, half, eng):
                key = (sq, half)
                if key not in mask_t:
                    mask_t[key] = maskp.tile(
                        [128, NSK // 2, SQT],
                        BF16,
                        tag="mask",
                        name=f"mask_{sq}_{half}",
                    )
                lo = half * (NSK // 2)
                eng.dma_start(
                    mask_t[key][:], maskT[:, sq, lo : lo + NSK // 2, :]
                )

            # ---------------- K projection, e-major ----------------
            with tc.tile_pool(name="ps_k", bufs=1, space="PSUM") as ps_k:
                kps = [
                    [
                        ps_k.tile(
                            [128, SQT], F32, tag=f"kps_{sq}_{c}", name=f"kps_{sq}_{c}"
                        )
                        for c in range(2)
                    ]
                    for sq in range(NSQ)
                ]
                # sync queue, priority order.  xq1-3 and masks 2/3 are
                # emitted later (inside the attention loop): the scheduler
                # expresses DMA deps as cumulative per-hw-queue counters, so
                # anything emitted after a DMA inherits its completion —
                # late bulk DMAs must be emitted after the early-critical
                # compute.
                nc.sync.dma_start(wk_sb[:], wkT[:])
                for e in range(EC):
                    nc.sync.dma_start(xk_all[:, e, :], xkT[:, e, :])
                nc.sync.dma_start(wq_sb[:], wqT[:])
                nc.sync.dma_start(xq_t[0][:], xqT[:, 0, :, :])
                nc.sync.dma_start(wv_sb[:], wvT[:])
                mask_dma(0, 0, nc.sync)
                mask_dma(0, 1, nc.sync)
                mask_dma(1, 0, nc.sync)
                mask_dma(1, 1, nc.sync)

                for e in range(EC):
                    for sq in range(NSQ):
                        ssl = slice(sq * SQT, (sq + 1) * SQT)
                        for c in range(2):
                            nc.tensor.matmul(
                                kps[sq][c][:],
                                wk_sb[:, e, c * 128 : (c + 1) * 128],
                                xk_all[:, e, ssl],
                                start=(e == 0),
                                stop=(e == EC - 1),
                            )

                # drain: per (sq,c) a dup staging tile ktc[p, j, s]
                # (j = head-within-chunk, K duplicated into both partition
                # halves like qt2) built from two bias-fused DVE writes plus
                # two 1KB-segment SBUF->SBUF spread DMAs; kd diag halves are
                # then plain aligned DVE copies — no 128B-segment DMAs.
                ktcs = {}

                def stage_k(sq, c):
                    t_ = ktcp.tile(
                        [128, 2, SQT], BF16, tag="ktc", name=f"ktc_{sq}_{c}"
                    )
                    ktcs[sq, c] = t_
                    ps = kps[sq][c]
                    nc.vector.tensor_scalar_add(
                        t_[0:64, 0, :], ps[0:64, :], bk_sb[0:64, c : c + 1]
                    )
                    nc.vector.tensor_scalar_add(
                        t_[64:128, 1, :], ps[64:128, :], bk_sb[64:128, c : c + 1]
                    )
                    nc.gpsimd.dma_start(t_[64:128, 0, :], t_[0:64, 0, :])
                    nc.gpsimd.dma_start(t_[0:64, 1, :], t_[64:128, 1, :])

                def kd_copies(c, j):
                    h = 2 * c + j
                    for sq in range(NSQ):
                        kv = ktcs[sq, c].rearrange(
                            "p j (t x) -> p j t x", t=4
                        )
                        nc.vector.tensor_copy(
                            kd[sq][0:64, h * 4 : h * 4 + 4, 0:64],
                            kv[0:64, j, :, 0:64],
                        )
                        nc.vector.tensor_copy(
                            kd[sq][64:128, h * 4 : h * 4 + 4, 64:128],
                            kv[64:128, j, :, 64:128],
                        )

                # Q0 and the first V block recycle the K-chain PSUM tags so
                # the PE keeps streaming while the drain runs on DVE/DMA
                qt2_first = [None]

                def emit_q0():
                    qt2 = qt2p.tile([128, NH, SQT], BF16, tag="qt2", name="qt2_0")
                    qt2_first[0] = qt2
                    for c in range(2):
                        ps = ps_k.tile(
                            [128, SQT], F32, tag=f"kps_0_{c}", name=f"q0ps_{c}"
                        )
                        for e in range(EC):
                            nc.tensor.matmul(
                                ps[:],
                                wq_sb[:, e, c * 128 : (c + 1) * 128],
                                xq_t[0][:, e, :],
                                start=(e == 0),
                                stop=(e == EC - 1),
                            )
                        qtc = kcp.tile([128, SQT], BF16, tag="kc", name="qtc0")
                        nc.vector.tensor_scalar_add(
                            qtc[:], ps[:], bq_sb[:, c : c + 1]
                        )
                        hA, hB = 2 * c, 2 * c + 1
                        nc.gpsimd.dma_start(qt2[0:64, hA, :], qtc[0:64, :])
                        nc.gpsimd.dma_start(qt2[64:128, hA, :], qtc[0:64, :])
                        nc.gpsimd.dma_start(qt2[0:64, hB, :], qtc[64:128, :])
                        nc.gpsimd.dma_start(qt2[64:128, hB, :], qtc[64:128, :])

                def emit_v0(sq, j, tag):
                    jsl = slice(sq * SQT + j * 128, sq * SQT + (j + 1) * 128)
                    ps = ps_k.tile([128, SQT], F32, tag=tag, name="v0ps")
                    psv = ps.rearrange("p (a c) -> p a c", c=CH)
                    for e in range(EC):
                        nc.tensor.matmul(
                            psv[:, 0, :],
                            xk_all[:, e, jsl],
                            wv_sb[:, e, :],
                            start=(e == 0),
                            stop=False,
                        )
                    nc.tensor.matmul(
                        psv[:, 0, :], ones_r[:], bv_sb[:], start=False, stop=True
                    )
                    nc.vector.tensor_copy(
                        vaug_sb[sq][:, j, :, 0:DH],
                        ps.rearrange("p (x h d) -> p x h d", h=NH, d=DH)[:, 0, :, :],
                    )

                stage_k(0, 0)
                stage_k(0, 1)
                emit_q0()
                stage_k(1, 0)
                stage_k(2, 0)
                stage_k(3, 0)
                kd_copies(0, 0)
                kd_copies(0, 1)
                stage_k(1, 1)
                stage_k(2, 1)
                for j in range(4):
                    emit_v0(0, j, f"kps_{1 + j // 2}_{j % 2}")
                stage_k(3, 1)

            # ---------------- attention + V/Q interleaved ----------------
            with (
                tc.tile_pool(name="ps_pq", bufs=2, space="PSUM") as ps_pq,
                tc.tile_pool(name="ps_s", bufs=2, space="PSUM") as ps_s,
                tc.tile_pool(name="ps_z", bufs=1, space="PSUM") as ps_z,
                tc.tile_pool(name="ps_t", bufs=1, space="PSUM") as ps_t,
            ):
                def emit_q(sq):
                    qt2 = qt2p.tile(
                        [128, NH, SQT], BF16, tag="qt2", name=f"qt2_{sq}"
                    )
                    for c in range(2):
                        ps = ps_pq.tile([128, SQT], F32, tag="projq", name="psq")
                        for e in range(EC):
                            nc.tensor.matmul(
                                ps[:],
                                wq_sb[:, e, c * 128 : (c + 1) * 128],
                                xq_t[sq][:, e, :],
                                start=(e == 0),
                                stop=(e == EC - 1),
                            )
                        qtc = kcp.tile([128, SQT], BF16, tag="kc", name="qtc")
                        nc.vector.tensor_scalar_add(
                            qtc[:], ps[:], bq_sb[:, c : c + 1]
                        )
                        hA, hB = 2 * c, 2 * c + 1
                        nc.gpsimd.dma_start(qt2[0:64, hA, :], qtc[0:64, :])
                        nc.gpsimd.dma_start(qt2[64:128, hA, :], qtc[0:64, :])
                        nc.gpsimd.dma_start(qt2[0:64, hB, :], qtc[64:128, :])
                        nc.gpsimd.dma_start(qt2[64:128, hB, :], qtc[64:128, :])
                    return qt2

                def emit_v(sq, j):
                    jsl = slice(sq * SQT + j * 128, sq * SQT + (j + 1) * 128)
                    ps = ps_s.tile([128, 2 * SQT], F32, tag="sc", name="psv")
                    psv = ps.rearrange("p (a c) -> p a c", c=CH)
                    for e in range(EC):
                        nc.tensor.matmul(
                            psv[:, 0, :],
                            xk_all[:, e, jsl],
                            wv_sb[:, e, :],
                            start=(e == 0),
                            stop=False,
                        )
                    nc.tensor.matmul(
                        psv[:, 0, :], ones_r[:], bv_sb[:], start=False, stop=True
                    )
                    nc.vector.tensor_copy(
                        vaug_sb[sq][:, j, :, 0:DH],
                        ps.rearrange("p (x h d) -> p x h d", h=NH, d=DH)[:, 0, :, :],
                    )

                def emit_scores(sq, h, qt2, probs):
                    # 2-chunk tiles: [128, 1024] 2D APs amortize ACT/DVE
                    # per-op overhead; all-bf16 multiply hits the DVE 2x mode
                    probs_v = probs.rearrange("p (a u) q -> p a (u q)", u=2)
                    mask_h = [
                        mask_t[(sq, 0)].rearrange("p (a u) q -> p a (u q)", u=2),
                        mask_t[(sq, 1)].rearrange("p (a u) q -> p a (u q)", u=2),
                    ]
                    for t2 in range(NSK // 2):
                        ps = ps_s.tile([128, 2 * SQT], F32, tag="sc", name="pssc")
                        for u in range(2):
                            t = 2 * t2 + u
                            nc.tensor.matmul(
                                ps[:, u * SQT : (u + 1) * SQT],
                                kd[t // 4][:, h * 4 + t % 4, :],
                                qt2[:, h, :],
                                start=True,
                                stop=True,
                            )
                        esc = escp.tile([128, 2 * SQT], BF16, tag="esc", name="esc")
                        nc.scalar.activation(esc[:], ps[:], AF.Exp, scale=0.125)
                        nc.vector.tensor_tensor(
                            probs_v[:, t2, :],
                            esc[:],
                            mask_h[t2 // 4][:, t2 % 4, :],
                            ALU.mult,
                        )

                def emit_av(sq, h, z_sb, probs):
                    zps = ps_z.tile([DH + 1, SQT], F32, tag="zps", name="zps")
                    for t in range(NSK):
                        nc.tensor.matmul(
                            zps[:],
                            vaug_sb[t // 4][:, t % 4, h, :],
                            probs[:, t, :],
                            start=(t == 0),
                            stop=(t == NSK - 1),
                        )
                    zaug = zaugp.tile([DH + 1, SQT], F32, tag="zaug", name="zaug")
                    nc.vector.tensor_copy(zaug[:], zps[:])
                    tps = ps_t.tile([128, 4, DH + 1], F32, tag="tps", name="tps")
                    for j in range(4):
                        nc.tensor.transpose(
                            tps[:, j, :],
                            zaug[:, j * 128 : (j + 1) * 128],
                            ident[: DH + 1, : DH + 1],
                        )
                    recip = recipp.tile([128, 4], F32, tag="recip", name="recip")
                    nc.vector.reciprocal(recip[:], tps[:, :, DH])
                    for j in range(4):
                        nc.vector.tensor_scalar_mul(
                            z_sb[:, j, h * DH : (h + 1) * DH],
                            tps[:, j, 0:DH],
                            recip[:, j : j + 1],
                        )

                # software pipeline: head h's scores ahead of head h-1's AV;
                # V-proj chains slot in after the first two score blocks and
                # Q(sq+1) after each sq's last score block.
                qt2_cur = qt2_first[0]
                prev = None
                for sq in range(NSQ):
                    z_sb = zp.tile([128, 4, CH], F32, tag="z", name="z_sb")
                    for h in range(NH):
                        probs = probsp.tile(
                            [128, NSK, SQT], BF16, tag="probs", name=f"probs_{sq}_{h}"
                        )
                        emit_scores(sq, h, qt2_cur, probs)
                        if sq == 0 and h == 0:
                            nc.sync.dma_start(xq_t[1][:], xqT[:, 1, :, :])
                            mask_dma(2, 0, nc.sync)
                            mask_dma(2, 1, nc.sync)
                            kd_copies(1, 0)
                            for j in range(4):
                                emit_v(1, j)
                        elif sq == 0 and h == 1:
                            nc.sync.dma_start(xq_t[2][:], xqT[:, 2, :, :])
                            nc.sync.dma_start(xq_t[3][:], xqT[:, 3, :, :])
                            mask_dma(3, 0, nc.sync)
                            mask_dma(3, 1, nc.sync)
                            kd_copies(1, 1)
                            for s2 in (2, 3):
                                for j in range(4):
                                    emit_v(s2, j)
                        if prev is not None:
                            emit_av(*prev)
                            if prev[1] == NH - 1:
                                psq, pz = prev[0], prev[2]
                                nc.gpsimd.dma_start(
                                    out[psq].rearrange("j p c -> p j c"), pz[:]
                                )
                        prev = (sq, h, z_sb, probs)
                    if sq + 1 < NSQ:
                        qt2_next = emit_q(sq + 1)
                        qt2_cur = qt2_next
                emit_av(*prev)
                nc.gpsimd.dma_start(
                    out[prev[0]].rearrange("j p c -> p j c"), prev[2][:]
                )

    return nc


_NC_CACHE = {}


def _get_nc():
    if "nc" not in _NC_CACHE:
        _NC_CACHE["nc"] = _build_nc()
    return _NC_CACHE["nc"]


def _prep_in_maps(x_q, x_k_v, attn_mask, w_q, b_q, w_k, b_k, w_v, b_v):
    import ml_dtypes

    bf16 = ml_dtypes.bfloat16
    x_q = np.asarray(x_q, dtype=np.float32)
    x_k_v = np.asarray(x_k_v, dtype=np.float32)
    attn_mask = np.asarray(attn_mask)
    w_q = np.asarray(w_q, dtype=np.float32)
    w_k = np.asarray(w_k, dtype=np.float32)
    w_v = np.asarray(w_v, dtype=np.float32)
    b_q = np.asarray(b_q, dtype=np.float32)
    b_k = np.asarray(b_k, dtype=np.float32)
    b_v = np.asarray(b_v, dtype=np.float32)

    # p-major layouts: every device DMA reads a contiguous per-partition
    # block (small strided segments run at ~35GB/s vs ~330GB/s contiguous)
    xqT = [
        np.ascontiguousarray(
            x_q[b].T.reshape(EC, 128, NSQ, SQT).transpose(1, 2, 0, 3)
        ).astype(bf16)
        for b in range(B)
    ]
    xkT = [
        np.ascontiguousarray(x_k_v[b].T.reshape(EC, 128, S).transpose(1, 0, 2))
        .astype(bf16)
        for b in range(B)
    ]
    maskT = [
        np.ascontiguousarray(
            (~attn_mask[b]).T.reshape(NSK, 128, NSQ, SQT).transpose(1, 2, 0, 3)
        ).astype(bf16)
        for b in range(B)
    ]
    def _wprep(w, g):
        wt = w[g * CH : (g + 1) * CH].T.reshape(EC, 128, CH).transpose(1, 0, 2)
        return np.ascontiguousarray(wt).astype(bf16)
    wqT = [_wprep(w_q, g) for g in range(HG)]
    wkT = [_wprep(w_k, g) for g in range(HG)]
    wvT = [_wprep(w_v, g) for g in range(HG)]
    # bias columns [128, 2]: bqc[p, c] = b_q[g*CH + c*128 + p]
    bqc = [
        np.ascontiguousarray(b_q[g * CH : (g + 1) * CH].reshape(2, 128).T)
        for g in range(HG)
    ]
    bkc = [
        np.ascontiguousarray(b_k[g * CH : (g + 1) * CH].reshape(2, 128).T)
        for g in range(HG)
    ]
    bvs = [
        b_v[g * CH : (g + 1) * CH].reshape(1, CH).astype(bf16) for g in range(HG)
    ]
    ones_row = np.ones((1, 128), dtype=bf16)
    ones_col = np.ones((128, NSK, NH), dtype=bf16)
    zkd = np.zeros((128, NH * 4 * 128), dtype=bf16)

    in_maps = []
    for core in range(NCORES):
        b, g = divmod(core, HG)
        in_maps.append(
            {
                "xqT": xqT[b],
                "xkT": xkT[b],
                "maskT": maskT[b],
                "wqT": wqT[g],
                "wkT": wkT[g],
                "wvT": wvT[g],
                "bqc": bqc[g],
                "bkc": bkc[g],
                "bv": bvs[g],
                "ones_row": ones_row,
                "ones_col": ones_col,
                "zkd": zkd,
            }
        )
    return in_maps


def _run(inputs, **runner_kwargs):
    nc = _get_nc()
    in_maps = _prep_in_maps(**inputs)
    res = run_bass_kernel_spmd(nc, in_maps, list(range(NCORES)), **runner_kwargs)
    z = np.empty((B, S, H * DH), dtype=np.float32)
    for core in range(NCORES):
        b, g = divmod(core, HG)
        z[b, :, g * CH : (g + 1) * CH] = res.results[core]["out"].reshape(S, CH)
    return z, res


def kernel(**inputs) -> np.ndarray:
    z, _ = _run(inputs)
    return z


# revision 8
# speedup vs baseline: 1.0833x; 1.0529x over previous
"""Trainium2 Bass kernel for nn_MultiHeadAttention_5162550690632 (v2).

B=2, S=2048, EMB=1024, H=16 heads x 64 dim. Sharding: 8 cores =
2 batches x 4 head-groups (4 heads each); every shard is independent
(tensor parallel on heads + data parallel on batch), no collectives.

v2 changes vs baseline (275.9us):
  * All operands bf16 (PSUM stays fp32, biases fp32): halves input DMA
    (x 16MB->8MB, w 3MB->1.5MB) and SBUF; host-sim rel err 7.7e-3 vs
    the 2e-2 gate.
  * K projection runs e-major across all 4 sq blocks (8 concurrent
    PSUM accumulation groups) so the first matmul starts as soon as the
    first 192KB of wk/xk land instead of after ~6MB (33us startup stall
    in the baseline trace).
  * kd (block-diagonal K tiles for the scores matmul) is written
    directly from the K-proj PSUM drain: one bias-fused DVE copy per
    (sq,c) plus 4 strided SBUF->SBUF DMAs, replacing 192 DVE casts
    (~48us of DVE busy) in the baseline.
  * Attention is software-pipelined immediately after K+Q(0): scores
    (sq0,h0) starts ~33us in; the 16 V-projection chains and Q(sq1..3)
    are interleaved into the scores/AV pipeline so the PE never idles
    while ACT (exp) streams.
  * Per-head normalize batches the 4 transposes into one PSUM tile and
    uses a single strided reciprocal.
"""

import numpy as np

import concourse.bass as bass
import concourse.mybir as mybir
import concourse.tile as tile
from concourse.tile import ScopedClock
from concourse.bass_utils import run_bass_kernel_spmd
from concourse.masks import make_identity

# ---------------------------------------------------------------------------
# Workaround: this neuronxcc rejects >1 sync wait on several instruction
# encodings ("Too many sync wait commands", CoreV3GenImpl setupSyncWait).
# TileContext attaches multiple waits per instruction and its exit drain
# waits on every live processor.  Split every extra wait into a dedicated
# single-wait NOP on the same engine right before the instruction —
# per-engine queues are in-order, so this is semantically identical.

_MAX_WAITS = 1


def _legalize_multi_waits(tc):
    nc = tc.nc
    for fn in nc.m.functions:
        for bb in fn.blocks:
            snapshot = list(bb.instructions)
            if not any(
                inst.sync_info is not None
                and len(inst.sync_info.on_wait) > _MAX_WAITS
                for inst in snapshot
            ):
                continue
            created = []
            new_list = []
            for inst in snapshot:
                si = inst.sync_info
                if si is not None and len(si.on_wait) > _MAX_WAITS:
                    waits = list(si.on_wait)
                    for w in waits[_MAX_WAITS:]:
                        nop = nc.engines[inst.engine].nop(
                            nofuse=True, hint="wait_split"
                        )
                        nop.ins.sync_info = mybir.SyncInfo(
                            on_wait=[w], on_update=[]
                        )
                        created.append(nop.ins.name)
                        new_list.append(nop.ins)
                    inst.sync_info = mybir.SyncInfo(
                        on_wait=waits[:_MAX_WAITS], on_update=list(si.on_update)
                    )
                new_list.append(inst)
            cur = nc.cur_bb.bb if hasattr(nc.cur_bb, "bb") else nc.cur_bb
            if cur is not None and cur.name != bb.name:
                cur.instructions = [
                    i for i in cur.instructions if i.name not in created
                ]
            bb.instructions = new_list


def _patched_drain_and_barrier(self, tick_clock, wait_clock):
    nc = self.nc
    probe = nc.sync.nop(nofuse=True, hint="drain_probe")
    wait_clock.add_sem_waits(probe.ins, ScopedClock({None: tick_clock.global_clock}))
    waits = list(probe.ins.sync_info.on_wait)
    probe.ins.sync_info = mybir.SyncInfo(on_wait=[], on_update=[])
    name2sem = {s.name: s for s in self.sems.allocated().values()}
    for w in waits:
        nc.sync.wait_ge(name2sem[w.ant_name], w.wait_value)
    _legalize_multi_waits(self)
    nc.sync.drain()
    nc.all_engine_barrier()
    popped = nc._tile_sem_poison_stack.pop()
    assert popped is self._sem_poison
    nc.clear_and_free_semaphores(list(self.sems.allocated().values()))
    nc.all_engine_barrier()


tile.TileContext._drain_and_barrier = _patched_drain_and_barrier

# ---------------------------------------------------------------------------

F32 = mybir.dt.float32
BF16 = mybir.dt.bfloat16
AF = mybir.ActivationFunctionType
ALU = mybir.AluOpType

B, S, EMB = 2, 2048, 1024
H, DH = 16, 64
NCORES = 8
HG = 4                      # head-groups
NH = H // HG                # heads per core = 4
CH = NH * DH                # channels per core = 256
EC = EMB // 128             # EMB chunks = 8
SQT = 512                   # q-tile width
NSQ = S // SQT              # 4
NSK = S // 128              # 16 sk chunks


def _build_nc():
    nc = bass.Bass()

    xqT = nc.declare_dram_parameter("xqT", [128, NSQ, EC, SQT], BF16, isOutput=False)
    xkT = nc.declare_dram_parameter("xkT", [128, EC, S], BF16, isOutput=False)
    wqT = nc.declare_dram_parameter("wqT", [128, EC, CH], BF16, isOutput=False)
    wkT = nc.declare_dram_parameter("wkT", [128, EC, CH], BF16, isOutput=False)
    wvT = nc.declare_dram_parameter("wvT", [128, EC, CH], BF16, isOutput=False)
    bqc = nc.declare_dram_parameter("bqc", [128, 2], F32, isOutput=False)
    bkc = nc.declare_dram_parameter("bkc", [128, 2], F32, isOutput=False)
    bv = nc.declare_dram_parameter("bv", [1, CH], BF16, isOutput=False)
    maskT = nc.declare_dram_parameter("maskT", [128, NSQ, NSK, SQT], BF16, isOutput=False)
    ones_row = nc.declare_dram_parameter("ones_row", [1, 128], BF16, isOutput=False)
    ones_col = nc.declare_dram_parameter(
        "ones_col", [128, NSK, NH], BF16, isOutput=False
    )
    zkd = nc.declare_dram_parameter("zkd", [128, NH * 4 * 128], BF16, isOutput=False)
    out = nc.declare_dram_parameter("out", [NSQ, 4, 128, CH], F32, isOutput=True)

    with tile.TileContext(nc) as tc:
        with (
            tc.tile_pool(name="persist", bufs=1) as persist,
            tc.tile_pool(name="xqp", bufs=2) as xqp,
            tc.tile_pool(name="qt2p", bufs=2) as qt2p,
            tc.tile_pool(name="maskp", bufs=4) as maskp,
            tc.tile_pool(name="probsp", bufs=3) as probsp,
            tc.tile_pool(name="kcp", bufs=2) as kcp,
            tc.tile_pool(name="ktcp", bufs=16) as ktcp,
            tc.tile_pool(name="escp", bufs=2) as escp,
            tc.tile_pool(name="zaugp", bufs=2) as zaugp,
            tc.tile_pool(name="zp", bufs=2) as zp,
            tc.tile_pool(name="recipp", bufs=2) as recipp,
        ):
            # block-diag K tiles: kd[g][:, h*4+tl, :] covers sk block 4g+tl
            kd = [
                persist.tile(
                    [128, NH * 4, 128], BF16, tag=f"kd_{g}", name=f"kd_{g}"
                )
                for g in range(NSQ)
            ]
            vaug_sb = [
                persist.tile(
                    [128, 4, NH, DH + 1], BF16, tag=f"vaug_{g}", name=f"vaug_{g}"
                )
                for g in range(NSQ)
            ]
            wq_sb = persist.tile([128, EC, CH], BF16, tag="wq")
            wk_sb = persist.tile([128, EC, CH], BF16, tag="wk")
            wv_sb = persist.tile([128, EC, CH], BF16, tag="wv")
            xk_all = persist.tile([128, EC, S], BF16, tag="xk_all")
            ident = persist.tile([128, 128], F32, tag="ident")
            ones_r = persist.tile([1, 128], BF16, tag="ones_r")
            bq_sb = persist.tile([128, 2], F32, tag="bq")
            bk_sb = persist.tile([128, 2], F32, tag="bk")
            bv_sb = persist.tile([1, CH], BF16, tag="bv")

            make_identity(nc, ident[:])
            # warm the Exp activation table during startup so the first real
            # exp doesn't pay the ~1.3us ACT_TABLE_LOAD on the critical path
            warm = persist.tile([1, 8], BF16, tag="warm")
            nc.scalar.activation(warm[:], ident[0:1, 0:8], AF.Exp, scale=0.125)
            # smalls + zero-fills on the gpsimd queue
            nc.gpsimd.dma_start(ones_r[:], ones_row[:])
            nc.gpsimd.dma_start(bq_sb[:], bqc[:])
            nc.gpsimd.dma_start(bk_sb[:], bkc[:])
            nc.gpsimd.dma_start(bv_sb[:], bv[:])
            for g in range(NSQ):
                nc.gpsimd.dma_start(
                    kd[g].rearrange("p i x -> p (i x)"), zkd[:]
                )
                nc.gpsimd.dma_start(
                    vaug_sb[g][:, :, :, DH : DH + 1],
                    ones_col[:, 4 * g : 4 * g + 4, :, None],
                )

            xq_t = [
                xqp.tile([128, EC, SQT], BF16, tag="xq", name=f"xq_{sq}")
                for sq in range(NSQ)
            ]
            mask_t = {}

            def mask_dma(sq, half, eng):
                key = (sq, half)
                if key not in mask_t:
                    mask_t[key] = maskp.tile(
                        [128, NSK // 2, SQT],
                        BF16,
                        tag="mask",
                        name=f"mask_{sq}_{half}",
                    )
                lo = half * (NSK // 2)
                eng.dma_start(
                    mask_t[key][:], maskT[:, sq, lo : lo + NSK // 2, :]
                )

            # ---------------- K projection, e-major ----------------
            with tc.tile_pool(name="ps_k", bufs=1, space="PSUM") as ps_k:
                kps = [
                    [
                        ps_k.tile(
                            [128, SQT], F32, tag=f"kps_{sq}_{c}", name=f"kps_{sq}_{c}"
                        )
                        for c in range(2)
                    ]
                    for sq in range(NSQ)
                ]
                # sync queue, priority order.  xq1-3 and masks 2/3 are
                # emitted later (inside the attention loop): the scheduler
                # expresses DMA deps as cumulative per-hw-queue counters, so
                # anything emitted after a DMA inherits its completion —
                # late bulk DMAs must be emitted after the early-critical
                # compute.
                nc.sync.dma_start(wk_sb[:], wkT[:])
                for e in range(EC):
                    nc.sync.dma_start(xk_all[:, e, :], xkT[:, e, :])
                nc.sync.dma_start(wq_sb[:], wqT[:])
                nc.sync.dma_start(xq_t[0][:], xqT[:, 0, :, :])
                nc.sync.dma_start(wv_sb[:], wvT[:])
                mask_dma(0, 0, nc.sync)
                mask_dma(0, 1, nc.sync)
                mask_dma(1, 0, nc.sync)
                mask_dma(1, 1, nc.sync)

                for e in range(EC):
                    for sq in range(NSQ):
                        ssl = slice(sq * SQT, (sq + 1) * SQT)
                        for c in range(2):
                            nc.tensor.matmul(
                                kps[sq][c][:],
                                wk_sb[:, e, c * 128 : (c + 1) * 128],
                                xk_all[:, e, ssl],
                                start=(e == 0),
                                stop=(e == EC - 1),
                            )

                # drain runs on the otherwise-idle ACT engine (Identity
                # with the bias as a per-partition AP) so DVE stays free for
                # the first mask-mults; ktd holds the partition-swapped dup
                # (two 1KB-segment SBUF->SBUF DMAs); kd diag halves are then
                # plain aligned DVE copies.
                kcs = {}
                ktds = {}

                def stage_k(sq, c):
                    kc = ktcp.tile([128, SQT], BF16, tag="ktc", name=f"kc_{sq}_{c}")
                    ktd = ktcp.tile([128, SQT], BF16, tag="ktc", name=f"ktd_{sq}_{c}")
                    kcs[sq, c] = kc
                    ktds[sq, c] = ktd
                    nc.scalar.activation(
                        kc[:], kps[sq][c][:], AF.Identity, bias=bk_sb[:, c : c + 1]
                    )
                    nc.gpsimd.dma_start(ktd[64:128, :], kc[0:64, :])
                    nc.gpsimd.dma_start(ktd[0:64, :], kc[64:128, :])

                def kd_copies(c, j):
                    h = 2 * c + j
                    for sq in range(NSQ):
                        kv = kcs[sq, c].rearrange("p (t x) -> p t x", t=4)
                        dv = ktds[sq, c].rearrange("p (t x) -> p t x", t=4)
                        src_a = kv if j == 0 else dv
                        src_b = dv if j == 0 else kv
                        nc.vector.tensor_copy(
                            kd[sq][0:64, h * 4 : h * 4 + 4, 0:64],
                            src_a[0:64, :, 0:64],
                        )
                        nc.vector.tensor_copy(
                            kd[sq][64:128, h * 4 : h * 4 + 4, 64:128],
                            src_b[64:128, :, 64:128],
                        )

                # Q0 and the first V block recycle the K-chain PSUM tags so
                # the PE keeps streaming while the drain runs on DVE/DMA
                qt2_first = [None]

                def emit_q0():
                    qt2 = qt2p.tile([128, NH, SQT], BF16, tag="qt2", name="qt2_0")
                    qt2_first[0] = qt2
                    for c in range(2):
                        ps = ps_k.tile(
                            [128, SQT], F32, tag=f"kps_0_{c}", name=f"q0ps_{c}"
                        )
                        for e in range(EC):
                            nc.tensor.matmul(
                                ps[:],
                                wq_sb[:, e, c * 128 : (c + 1) * 128],
                                xq_t[0][:, e, :],
                                start=(e == 0),
                                stop=(e == EC - 1),
                            )
                        qtc = kcp.tile([128, SQT], BF16, tag="kc", name="qtc0")
                        nc.scalar.activation(
                            qtc[:], ps[:], AF.Identity, bias=bq_sb[:, c : c + 1]
                        )
                        hA, hB = 2 * c, 2 * c + 1
                        nc.sync.dma_start(qt2[0:64, hA, :], qtc[0:64, :])
                        nc.sync.dma_start(qt2[64:128, hA, :], qtc[0:64, :])
                        nc.sync.dma_start(qt2[0:64, hB, :], qtc[64:128, :])
                        nc.sync.dma_start(qt2[64:128, hB, :], qtc[64:128, :])

                def emit_v0(sq, j, tag):
                    jsl = slice(sq * SQT + j * 128, sq * SQT + (j + 1) * 128)
                    ps = ps_k.tile([128, SQT], F32, tag=tag, name="v0ps")
                    psv = ps.rearrange("p (a c) -> p a c", c=CH)
                    for e in range(EC):
                        nc.tensor.matmul(
                            psv[:, 0, :],
                            xk_all[:, e, jsl],
                            wv_sb[:, e, :],
                            start=(e == 0),
                            stop=False,
                        )
                    nc.tensor.matmul(
                        psv[:, 0, :], ones_r[:], bv_sb[:], start=False, stop=True
                    )
                    nc.scalar.activation(
                        vaug_sb[sq][:, j, :, 0:DH],
                        ps.rearrange("p (x h d) -> p x h d", h=NH, d=DH)[:, 0, :, :],
                        AF.Copy,
                    )

                stage_k(0, 0)
                stage_k(1, 0)
                stage_k(2, 0)
                stage_k(3, 0)
                stage_k(0, 1)
                emit_q0()
                kd_copies(0, 0)
                kd_copies(0, 1)
                stage_k(1, 1)
                stage_k(2, 1)
                for j in range(4):
                    emit_v0(0, j, f"kps_{1 + j // 2}_{j % 2}")
                stage_k(3, 1)

            # ---------------- attention + V/Q interleaved ----------------
            with (
                tc.tile_pool(name="ps_pq", bufs=2, space="PSUM") as ps_pq,
                tc.tile_pool(name="ps_s", bufs=2, space="PSUM") as ps_s,
                tc.tile_pool(name="ps_z", bufs=1, space="PSUM") as ps_z,
                tc.tile_pool(name="ps_t", bufs=1, space="PSUM") as ps_t,
            ):
                def emit_q(sq):
                    qt2 = qt2p.tile(
                        [128, NH, SQT], BF16, tag="qt2", name=f"qt2_{sq}"
                    )
                    for c in range(2):
                        ps = ps_pq.tile([128, SQT], F32, tag="projq", name="psq")
                        for e in range(EC):
                            nc.tensor.matmul(
                                ps[:],
                                wq_sb[:, e, c * 128 : (c + 1) * 128],
                                xq_t[sq][:, e, :],
                                start=(e == 0),
                                stop=(e == EC - 1),
                            )
                        qtc = kcp.tile([128, SQT], BF16, tag="kc", name="qtc")
                        nc.vector.tensor_scalar_add(
                            qtc[:], ps[:], bq_sb[:, c : c + 1]
                        )
                        hA, hB = 2 * c, 2 * c + 1
                        nc.gpsimd.dma_start(qt2[0:64, hA, :], qtc[0:64, :])
                        nc.gpsimd.dma_start(qt2[64:128, hA, :], qtc[0:64, :])
                        nc.gpsimd.dma_start(qt2[0:64, hB, :], qtc[64:128, :])
                        nc.gpsimd.dma_start(qt2[64:128, hB, :], qtc[64:128, :])
                    return qt2

                def emit_v(sq, j):
                    jsl = slice(sq * SQT + j * 128, sq * SQT + (j + 1) * 128)
                    ps = ps_s.tile([128, 2 * SQT], F32, tag="sc", name="psv")
                    psv = ps.rearrange("p (a c) -> p a c", c=CH)
                    for e in range(EC):
                        nc.tensor.matmul(
                            psv[:, 0, :],
                            xk_all[:, e, jsl],
                            wv_sb[:, e, :],
                            start=(e == 0),
                            stop=False,
                        )
                    nc.tensor.matmul(
                        psv[:, 0, :], ones_r[:], bv_sb[:], start=False, stop=True
                    )
                    nc.vector.tensor_copy(
                        vaug_sb[sq][:, j, :, 0:DH],
                        ps.rearrange("p (x h d) -> p x h d", h=NH, d=DH)[:, 0, :, :],
                    )

                def emit_scores(sq, h, qt2, probs):
                    # 2-chunk tiles: [128, 1024] 2D APs amortize ACT/DVE
                    # per-op overhead; all-bf16 multiply hits the DVE 2x mode
                    probs_v = probs.rearrange("p (a u) q -> p a (u q)", u=2)
                    mask_h = [
                        mask_t[(sq, 0)].rearrange("p (a u) q -> p a (u q)", u=2),
                        mask_t[(sq, 1)].rearrange("p (a u) q -> p a (u q)", u=2),
                    ]
                    for t2 in range(NSK // 2):
                        ps = ps_s.tile([128, 2 * SQT], F32, tag="sc", name="pssc")
                        for u in range(2):
                            t = 2 * t2 + u
                            nc.tensor.matmul(
                                ps[:, u * SQT : (u + 1) * SQT],
                                kd[t // 4][:, h * 4 + t % 4, :],
                                qt2[:, h, :],
                                start=True,
                                stop=True,
                            )
                        esc = escp.tile([128, 2 * SQT], BF16, tag="esc", name="esc")
                        nc.scalar.activation(esc[:], ps[:], AF.Exp, scale=0.125)
                        nc.vector.tensor_tensor(
                            probs_v[:, t2, :],
                            esc[:],
                            mask_h[t2 // 4][:, t2 % 4, :],
                            ALU.mult,
                        )

                def emit_av(sq, h, z_sb, probs):
                    zps = ps_z.tile([DH + 1, SQT], F32, tag="zps", name="zps")
                    for t in range(NSK):
                        nc.tensor.matmul(
                            zps[:],
                            vaug_sb[t // 4][:, t % 4, h, :],
                            probs[:, t, :],
                            start=(t == 0),
                            stop=(t == NSK - 1),
                        )
                    zaug = zaugp.tile([DH + 1, SQT], F32, tag="zaug", name="zaug")
                    nc.vector.tensor_copy(zaug[:], zps[:])
                    tps = ps_t.tile([128, 4, DH + 1], F32, tag="tps", name="tps")
                    for j in range(4):
                        nc.tensor.transpose(
                            tps[:, j, :],
                            zaug[:, j * 128 : (j + 1) * 128],
                            ident[: DH + 1, : DH + 1],
                        )
                    recip = recipp.tile([128, 4], F32, tag="recip", name="recip")
                    nc.vector.reciprocal(recip[:], tps[:, :, DH])
                    for j in range(4):
                        nc.vector.tensor_scalar_mul(
                            z_sb[:, j, h * DH : (h + 1) * DH],
                            tps[:, j, 0:DH],
                            recip[:, j : j + 1],
                        )

                # software pipeline: head h's scores ahead of head h-1's AV;
                # V-proj chains slot in after the first two score blocks and
                # Q(sq+1) after each sq's last score block.
                qt2_cur = qt2_first[0]
                prev = None
                for sq in range(NSQ):
                    z_sb = zp.tile([128, 4, CH], F32, tag="z", name="z_sb")
                    for h in range(NH):
                        probs = probsp.tile(
                            [128, NSK, SQT], BF16, tag="probs", name=f"probs_{sq}_{h}"
                        )
                        emit_scores(sq, h, qt2_cur, probs)
                        if sq == 0 and h == 0:
                            nc.sync.dma_start(xq_t[1][:], xqT[:, 1, :, :])
                            mask_dma(2, 0, nc.sync)
                            mask_dma(2, 1, nc.sync)
                            kd_copies(1, 0)
                            for j in range(4):
                                emit_v(1, j)
                        elif sq == 0 and h == 1:
                            nc.sync.dma_start(xq_t[2][:], xqT[:, 2, :, :])
                            nc.sync.dma_start(xq_t[3][:], xqT[:, 3, :, :])
                            mask_dma(3, 0, nc.sync)
                            mask_dma(3, 1, nc.sync)
                            kd_copies(1, 1)
                            for s2 in (2, 3):
                                for j in range(4):
                                    emit_v(s2, j)
                        if prev is not None:
                            emit_av(*prev)
                            if prev[1] == NH - 1:
                                psq, pz = prev[0], prev[2]
                                nc.gpsimd.dma_start(
                                    out[psq].rearrange("j p c -> p j c"), pz[:]
                                )
                        prev = (sq, h, z_sb, probs)
                    if sq + 1 < NSQ:
                        qt2_next = emit_q(sq + 1)
                        qt2_cur = qt2_next
                emit_av(*prev)
                nc.gpsimd.dma_start(
                    out[prev[0]].rearrange("j p c -> p j c"), prev[2][:]
                )

    return nc


_NC_CACHE = {}


def _get_nc():
    if "nc" not in _NC_CACHE:
        _NC_CACHE["nc"] = _build_nc()
    return _NC_CACHE["nc"]


def _prep_in_maps(x_q, x_k_v, attn_mask, w_q, b_q, w_k, b_k, w_v, b_v):
    import ml_dtypes

    bf16 = ml_dtypes.bfloat16
    x_q = np.asarray(x_q, dtype=np.float32)
    x_k_v = np.asarray(x_k_v, dtype=np.float32)
    attn_mask = np.asarray(attn_mask)
    w_q = np.asarray(w_q, dtype=np.float32)
    w_k = np.asarray(w_k, dtype=np.float32)
    w_v = np.asarray(w_v, dtype=np.float32)
    b_q = np.asarray(b_q, dtype=np.float32)
    b_k = np.asarray(b_k, dtype=np.float32)
    b_v = np.asarray(b_v, dtype=np.float32)

    # p-major layouts: every device DMA reads a contiguous per-partition
    # block (small strided segments run at ~35GB/s vs ~330GB/s contiguous)
    xqT = [
        np.ascontiguousarray(
            x_q[b].T.reshape(EC, 128, NSQ, SQT).transpose(1, 2, 0, 3)
        ).astype(bf16)
        for b in range(B)
    ]
    xkT = [
        np.ascontiguousarray(x_k_v[b].T.reshape(EC, 128, S).transpose(1, 0, 2))
        .astype(bf16)
        for b in range(B)
    ]
    maskT = [
        np.ascontiguousarray(
            (~attn_mask[b]).T.reshape(NSK, 128, NSQ, SQT).transpose(1, 2, 0, 3)
        ).astype(bf16)
        for b in range(B)
    ]
    def _wprep(w, g):
        wt = w[g * CH : (g + 1) * CH].T.reshape(EC, 128, CH).transpose(1, 0, 2)
        return np.ascontiguousarray(wt).astype(bf16)
    wqT = [_wprep(w_q, g) for g in range(HG)]
    wkT = [_wprep(w_k, g) for g in range(HG)]
    wvT = [_wprep(w_v, g) for g in range(HG)]
    # bias columns [128, 2]: bqc[p, c] = b_q[g*CH + c*128 + p]
    bqc = [
        np.ascontiguousarray(b_q[g * CH : (g + 1) * CH].reshape(2, 128).T)
        for g in range(HG)
    ]
    bkc = [
        np.ascontiguousarray(b_k[g * CH : (g + 1) * CH].reshape(2, 128).T)
        for g in range(HG)
    ]
    bvs = [
        b_v[g * CH : (g + 1) * CH].reshape(1, CH).astype(bf16) for g in range(HG)
    ]
    ones_row = np.ones((1, 128), dtype=bf16)
    ones_col = np.ones((128, NSK, NH), dtype=bf16)
    zkd = np.zeros((128, NH * 4 * 128), dtype=bf16)

    in_maps = []
    for core in range(NCORES):
        b, g = divmod(core, HG)
        in_maps.append(
            {
                "xqT": xqT[b],
                "xkT": xkT[b],
                "maskT": maskT[b],
                "wqT": wqT[g],
                "wkT": wkT[g],
                "wvT": wvT[g],
                "bqc": bqc[g],
                "bkc": bkc[g],
                "bv": bvs[g],
                "ones_row": ones_row,
                "ones_col": ones_col,
                "zkd": zkd,
            }
        )
    return in_maps


def _run(inputs, **runner_kwargs):
    nc = _get_nc()
    in_maps = _prep_in_maps(**inputs)
    res = run_bass_kernel_spmd(nc, in_maps, list(range(NCORES)), **runner_kwargs)
    z = np.empty((B, S, H * DH), dtype=np.float32)
    for core in range(NCORES):
        b, g = divmod(core, HG)
        z[b, :, g * CH : (g + 1) * CH] = res.results[core]["out"].reshape(S, CH)
    return z, res


def kernel(**inputs) -> np.ndarray:
    z, _ = _run(inputs)
    return z
